# revision 19
# baseline (speedup 1.0000x reference)
"""Conv2d 3x3 (stride 1, pad 1) on Trainium2, data-parallel over batch.

Full problem: x [16, 32, 512, 512] f32, kernels [32, 32, 3, 3] f32
-> out [16, 32, 512, 512] f32.

Sharding: batch 16 / 8 cores = 2 images per core; kernels replicated.
No collectives needed.

Per-core kernel strategy (memory-bound target):
- Conv expressed as 9 accumulating 32x32 matmuls (one per tap) into PSUM.
- The 128x128 PE array is addressed as 16 concurrent 32x32 sub-arrays via
  tile_position: row group i = image band i (4 horizontal bands), col
  group j = output-row slot j (4 rows in flight per band).
- Input rows live in SBUF as [32 ch, (R+2) x (W+2)] with zero-padded
  columns, so every tap (kh, kw) is just a free-dim offset: kh*514 + kw.
- PSUM tile [128, 512] per (band, step) holds 4 output rows (one per col
  group); evicted to SBUF by Vector/Scalar copies, DMA'd out with a
  scatter access pattern.
"""

import numpy as np
from contextlib import ExitStack

import concourse.bass as bass
import concourse.mybir as mybir
import concourse.tile as tile
from concourse.bass_utils import run_bass_kernel_spmd

from concourse.vector_clock import ScopedClock

F32 = mybir.dt.float32
KH = KW = 3
NBANDS = 4  # row groups = horizontal image bands
NCOLG = 4  # col groups = output rows in flight per band

# Full-problem geometry (hardcoded; kernel.py must be self-contained)
FULL_B, FULL_C, FULL_H, FULL_W = 16, 32, 512, 512
N_CORES = 8


def split_multi_waits(nc, cap=1):
    """This walrus build rejects instructions carrying more than `cap` sync
    wait commands ("Too many sync wait commands", setupSyncWait). Hoist
    excess waits onto single-wait NoOps inserted just before the instruction
    on the same engine queue (queues are in-order, so semantics are
    unchanged)."""
    n_split = 0
    for fn in nc.m.functions:
        for blk in fn.blocks:
            insts = blk.instructions
            if not any(
                i.sync_info is not None and len(i.sync_info.on_wait) > cap
                for i in insts
            ):
                continue
            new = []
            for inst in insts:
                si = inst.sync_info
                if si is not None and len(si.on_wait) > cap:
                    waits = list(si.on_wait)
                    n_split += 1
                    for k in range(0, len(waits) - cap, cap):
                        nop = mybir.InstNoOp(
                            name=nc.get_next_instruction_name(), ins=[], outs=[]
                        )
                        nop.engine = inst.engine
                        nop.sync_info = mybir.SyncInfo(
                            on_wait=waits[k : k + cap], on_update=[]
                        )
                        new.append(nop)
                    inst.sync_info = mybir.SyncInfo(
                        on_wait=waits[len(waits) - cap :],
                        on_update=list(si.on_update),
                    )
                new.append(inst)
            blk.instructions = new
    return n_split


def emit_conv(
    nc, tc, ctx, x_ap, w_ap, out_ap, B, C, H, W, R=16, prefix="", do_mm=True
):
    """Emit the Tile program for a per-core conv: x [B, C, H, W] (local
    batch), w [C, KH*KW*C] (pre-transposed on host: [ic, (kh kw oc)]),
    out [B, C, H, W]."""
    assert C == 32
    HB = H // NBANDS  # rows per band
    assert HB * NBANDS == H
    assert HB % R == 0
    T = HB // R  # rounds per image
    S = R // NCOLG  # steps per round (4 rows per step per band)
    assert S * NCOLG == R
    Wp = W + 2  # zero-padded row length
    assert W <= 512

    w_pool = ctx.enter_context(tc.tile_pool(name=prefix + "wpool", bufs=1))
    in_pool = ctx.enter_context(tc.tile_pool(name=prefix + "inpool", bufs=2))
    out_pool = ctx.enter_context(tc.tile_pool(name=prefix + "outpool", bufs=2 * NBANDS))
    psum_pool = ctx.enter_context(
        tc.tile_pool(name=prefix + "psumpool", bufs=2 * NBANDS, space="PSUM")
    )

    # Weights: replicate [32, 9*32] into each of the 4 partition groups so
    # lhsT.base_partition() matches the rhs row group.
    w_tile = w_pool.tile([128, KH * KW * C], F32, name=prefix + "w_tile", tag="w")
    for r in range(NBANDS):
        nc.sync.dma_start(out=w_tile[32 * r : 32 * r + 32, :], in_=w_ap[:, :])

    for b in range(B):
        for t in range(T):
            # ---- load input rows for this round: band i covers output rows
            # [i*HB + t*R, i*HB + t*R + R), needing input rows -1..R+1 around it.
            in_tile = in_pool.tile(
                [128, (R + 2) * Wp], F32, name=f"{prefix}in_{b}_{t}", tag="xin"
            )
            in_rows = in_tile.rearrange("p (r w) -> p r w", w=Wp)
            # zero the left/right pad columns for all row slots
            nc.vector.memset(in_rows[:, :, 0:1], 0.0)
            nc.vector.memset(in_rows[:, :, Wp - 1 : Wp], 0.0)
            for i in range(NBANDS):
                row0 = i * HB + t * R
                lo = max(row0 - 1, 0)
                hi = min(row0 + R + 1, H)
                slot0 = lo - (row0 - 1)
                cnt = hi - lo
                dst = in_rows[32 * i : 32 * i + 32, slot0 : slot0 + cnt, 1 : 1 + W]
                nc.sync.dma_start(out=dst, in_=x_ap[b, :, lo:hi, :])
                if row0 == 0:  # top image boundary: zero row slot 0
                    nc.vector.memset(in_rows[32 * i : 32 * i + 32, 0:1, :], 0.0)
                if row0 + R == H:  # bottom image boundary: zero last slot
                    nc.vector.memset(
                        in_rows[32 * i : 32 * i + 32, R + 1 : R + 2, :], 0.0
                    )

            out_tiles = []
            for i in range(NBANDS):
                ot = out_pool.tile(
                    [128, S * W], F32, name=f"{prefix}out_{b}_{t}_{i}", tag="osb"
                )
                out_tiles.append(ot)

            for s in range(S if do_mm else 0):
                psums = []
                for i in range(NBANDS):
                    pt = psum_pool.tile(
                        [128, W], F32, name=f"{prefix}ps_{b}_{t}_{s}_{i}", tag="acc"
                    )
                    psums.append(pt)
                # 9 taps, 16 concurrent 32x32 sub-array matmuls per tap
                for off in range(KH * KW):
                    kh, kw = off // KW, off % KW
                    for i in range(NBANDS):
                        lhsT = w_tile[32 * i : 32 * i + 32, off * C : off * C + C]
                        for j in range(NCOLG):
                            m = NCOLG * s + j  # local output row in round
                            fo = (m + kh) * Wp + kw
                            rhs = in_tile[32 * i : 32 * i + 32, fo : fo + W]
                            nc.tensor.matmul(
                                psums[i][32 * j : 32 * j + 32, :],
                                lhsT,
                                rhs,
                                start=(off == 0),
                                stop=(off == KH * KW - 1),
                                tile_position=(32 * i, 32 * j),
                                # 4 col groups share each bank (disjoint
                                # partition slices); the sim's group check
                                # is partition-coarse and false-positives.
                                skip_group_check=True,
                            )
                # evict: one [128, W] copy per band (4 rows each)
                for i in range(NBANDS):
                    dst = out_tiles[i][:, s * W : (s + 1) * W]
                    if i % 2 == 0:
                        nc.vector.tensor_copy(dst, psums[i][:, :])
                    else:
                        nc.scalar.copy(dst, psums[i][:, :])

            # ---- store: out_tile [128, S*W] partition 32j+c, free s*W+x
            # maps to out[b, c, row0 + 4s + j, x]
            for i in range(NBANDS):
                row0 = i * HB + t * R
                dstv = out_ap[b, :, row0 : row0 + R, :].rearrange(
                    "c (s j) x -> j c s x", s=S, j=NCOLG
                )
                for j in range(NCOLG):
                    src = out_tiles[i][32 * j : 32 * j + 32, :].rearrange(
                        "c (s x) -> c s x", x=W
                    )
                    nc.sync.dma_start(out=dstv[j], in_=src)


def build_conv_nc(B, C, H, W, R=16, passes=1, do_mm=True):
    nc = bass.Bass("TRN2", target_bir_lowering=False, debug=False)
    x = nc.declare_dram_parameter("x", [B, C, H, W], F32, isOutput=False)
    w = nc.declare_dram_parameter("kernels_t", [C, KH * KW * C], F32, isOutput=False)
    out = nc.declare_dram_parameter("out", [B, C, H, W], F32, isOutput=True)
    with tile.TileContext(nc) as tc:
        with ExitStack() as ctx:
            emit_conv(nc, tc, ctx, x[:], w[:], out[:], B, C, H, W, R=R, do_mm=do_mm)
        # extra timing-probe passes into a scratch DRAM tensor (own pool
        # scope so SBUF is reused)
        for p in range(1, passes):
            scratch = nc.dram_tensor(f"scratch{p}", [B, C, H, W], F32)
            with ExitStack() as ctx:
                emit_conv(
                    nc, tc, ctx, x[:], w[:], scratch[:], B, C, H, W, R=R,
                    prefix=f"p{p}_", do_mm=do_mm,
                )
    split_multi_waits(nc, cap=1)
    return nc


_NC_CACHE = {}


def _get_nc():
    key = (FULL_B // N_CORES, FULL_C, FULL_H, FULL_W)
    if key not in _NC_CACHE:
        _NC_CACHE[key] = build_conv_nc(*key)
    return _NC_CACHE[key]


def host_weights(kernels: np.ndarray) -> np.ndarray:
    # [oc, ic, kh, kw] -> [ic, (kh kw oc)] contiguous, so the weight DMA is
    # a plain 2D copy.
    return np.ascontiguousarray(kernels.transpose(1, 2, 3, 0).reshape(32, -1))


def kernel(x: np.ndarray, kernels: np.ndarray) -> np.ndarray:
    assert x.shape == (FULL_B, FULL_C, FULL_H, FULL_W), x.shape
    nc = _get_nc()
    bl = FULL_B // N_CORES
    wt = host_weights(np.asarray(kernels, dtype=np.float32))
    xs = np.asarray(x, dtype=np.float32)
    in_maps = [
        {"x": xs[i * bl : (i + 1) * bl], "kernels_t": wt} for i in range(N_CORES)
    ]
    res = run_bass_kernel_spmd(nc, in_maps, list(range(N_CORES))).results
    out = np.concatenate([res[i]["out"] for i in range(N_CORES)], axis=0)
    return out.astype(np.float32, copy=False)


# revision 23
# speedup vs baseline: 1.6695x; 1.6695x over previous
"""Conv2d 3x3 (stride 1, pad 1) on Trainium2, data-parallel over batch.

Full problem: x [16, 32, 512, 512] f32, kernels [32, 32, 3, 3] f32
-> out [16, 32, 512, 512] f32.

Sharding: batch 16 / 8 cores = 2 images per core; kernels replicated.
No collectives needed.

Per-core kernel strategy (memory-bound target):
- Conv expressed as 9 accumulating 32x32 matmuls (one per tap) into PSUM.
- The 128x128 PE array is addressed as 16 concurrent 32x32 sub-arrays via
  tile_position: row group i = image band i (4 horizontal bands), col
  group j = output-row slot j (4 rows in flight per band).
- Input rows live in SBUF as [32 ch, (R+2) x (W+2)] with zero-padded
  columns, so every tap (kh, kw) is just a free-dim offset: kh*514 + kw.
- PSUM tile [128, 512] per (band, step) holds 4 output rows (one per col
  group); evicted to SBUF by Vector/Scalar copies, DMA'd out with a
  scatter access pattern.
"""

import numpy as np
from contextlib import ExitStack

import concourse.bass as bass
import concourse.mybir as mybir
import concourse.tile as tile
from concourse.bass_utils import run_bass_kernel_spmd

F32 = mybir.dt.float32
KH = KW = 3
NBANDS = 4  # row groups = horizontal image bands
NCOLG = 4  # col groups = output rows in flight per band

# Full-problem geometry (hardcoded; kernel.py must be self-contained)
FULL_B, FULL_C, FULL_H, FULL_W = 16, 32, 512, 512
N_CORES = 8


def split_multi_waits(nc, cap=1):
    """This walrus build rejects instructions carrying more than `cap` sync
    wait commands ("Too many sync wait commands", setupSyncWait). Hoist
    excess waits onto single-wait NoOps inserted just before the instruction
    on the same engine queue (queues are in-order, so semantics are
    unchanged)."""
    n_split = 0
    for fn in nc.m.functions:
        for blk in fn.blocks:
            insts = blk.instructions
            if not any(
                i.sync_info is not None and len(i.sync_info.on_wait) > cap
                for i in insts
            ):
                continue
            new = []
            for inst in insts:
                si = inst.sync_info
                if si is not None and len(si.on_wait) > cap:
                    waits = list(si.on_wait)
                    n_split += 1
                    for k in range(0, len(waits) - cap, cap):
                        nop = mybir.InstNoOp(
                            name=nc.get_next_instruction_name(), ins=[], outs=[]
                        )
                        nop.engine = inst.engine
                        nop.sync_info = mybir.SyncInfo(
                            on_wait=waits[k : k + cap], on_update=[]
                        )
                        new.append(nop)
                    inst.sync_info = mybir.SyncInfo(
                        on_wait=waits[len(waits) - cap :],
                        on_update=list(si.on_update),
                    )
                new.append(inst)
            blk.instructions = new
    return n_split


def emit_conv(
    nc, tc, ctx, x_ap, w_ap, out_ap, B, C, H, W, R=16, prefix="", do_mm=True
):
    """Emit the Tile program for a per-core conv: x [B, C, H, W] (local
    batch), w [C, KH*KW*C] (pre-transposed on host: [ic, (kh kw oc)]),
    out [B, C, H, W]."""
    assert C == 32
    HB = H // NBANDS  # rows per band
    assert HB * NBANDS == H
    assert HB % R == 0
    T = HB // R  # rounds per image
    S = R // NCOLG  # steps per round (4 rows per step per band)
    assert S * NCOLG == R
    Wp = W + 2  # zero-padded row length
    assert W <= 512

    w_pool = ctx.enter_context(tc.tile_pool(name=prefix + "wpool", bufs=1))
    in_pool = ctx.enter_context(tc.tile_pool(name=prefix + "inpool", bufs=2))
    out_pool = ctx.enter_context(tc.tile_pool(name=prefix + "outpool", bufs=2 * NBANDS))
    psum_pool = ctx.enter_context(
        tc.tile_pool(name=prefix + "psumpool", bufs=2 * NBANDS, space="PSUM")
    )

    # Weights: replicate [32, 9*32] into each of the 4 partition groups so
    # lhsT.base_partition() matches the rhs row group.
    w_tile = w_pool.tile([128, KH * KW * C], F32, name=prefix + "w_tile", tag="w")
    for r in range(NBANDS):
        nc.sync.dma_start(out=w_tile[32 * r : 32 * r + 32, :], in_=w_ap[:, :])

    for b in range(B):
        for t in range(T):
            # ---- load input rows for this round: band i covers output rows
            # [i*HB + t*R, i*HB + t*R + R), needing input rows -1..R+1 around it.
            in_tile = in_pool.tile(
                [128, (R + 2) * Wp], F32, name=f"{prefix}in_{b}_{t}", tag="xin"
            )
            in_rows = in_tile.rearrange("p (r w) -> p r w", w=Wp)
            # zero the left/right pad columns for all row slots
            nc.vector.memset(in_rows[:, :, 0:1], 0.0)
            nc.vector.memset(in_rows[:, :, Wp - 1 : Wp], 0.0)
            for i in range(NBANDS):
                row0 = i * HB + t * R
                lo = max(row0 - 1, 0)
                hi = min(row0 + R + 1, H)
                slot0 = lo - (row0 - 1)
                cnt = hi - lo
                dst = in_rows[32 * i : 32 * i + 32, slot0 : slot0 + cnt, 1 : 1 + W]
                nc.sync.dma_start(out=dst, in_=x_ap[b, :, lo:hi, :])
                if row0 == 0:  # top image boundary: zero row slot 0
                    nc.vector.memset(in_rows[32 * i : 32 * i + 32, 0:1, :], 0.0)
                if row0 + R == H:  # bottom image boundary: zero last slot
                    nc.vector.memset(
                        in_rows[32 * i : 32 * i + 32, R + 1 : R + 2, :], 0.0
                    )

            out_tiles = []
            for i in range(NBANDS):
                ot = out_pool.tile(
                    [128, S * W], F32, name=f"{prefix}out_{b}_{t}_{i}", tag="osb"
                )
                if not do_mm:  # timing probe: mark tile written
                    nc.vector.memset(ot[:, 0:1], 0.0)
                out_tiles.append(ot)

            SG = 1  # steps sharing one weight load (2 broke tile scheduling)
            for sg in range(0, S if do_mm else 0, SG):
                psums = {}
                for s2 in range(SG):
                    for i in range(NBANDS):
                        pt = psum_pool.tile(
                            [128, W],
                            F32,
                            name=f"{prefix}ps_{b}_{t}_{sg + s2}_{i}",
                            tag="acc",
                        )
                        psums[(s2, i)] = pt
                # 9 taps; 16 concurrent 32x32 sub-array matmuls per tap; each
                # sub-array runs SG rows back-to-back on one weight load
                for off in range(KH * KW):
                    kh, kw = off // KW, off % KW
                    for i in range(NBANDS):
                        lhsT = w_tile[32 * i : 32 * i + 32, off * C : off * C + C]
                        for j in range(NCOLG):
                            for s2 in range(SG):
                                m = NCOLG * (sg + s2) + j  # local output row
                                fo = (m + kh) * Wp + kw
                                rhs = in_tile[32 * i : 32 * i + 32, fo : fo + W]
                                nc.tensor.matmul(
                                    psums[(s2, i)][32 * j : 32 * j + 32, :],
                                    lhsT,
                                    rhs,
                                    start=(off == 0),
                                    stop=(off == KH * KW - 1),
                                    tile_position=(32 * i, 32 * j),
                                    # 4 col groups share each bank (disjoint
                                    # partition slices); the sim's group check
                                    # is partition-coarse and false-positives.
                                    skip_group_check=True,
                                )
                # evict: one [128, W] copy per band per step (4 rows each)
                for s2 in range(SG):
                    for i in range(NBANDS):
                        dst = out_tiles[i][:, (sg + s2) * W : (sg + s2 + 1) * W]
                        if i % 2 == 0:
                            nc.vector.tensor_copy(dst, psums[(s2, i)][:, :])
                        else:
                            nc.scalar.copy(dst, psums[(s2, i)][:, :])

            # ---- store: out_tile [128, S*W] partition 32j+c, free s*W+x
            # maps to out[b, c, row0 + 4s + j, x]
            for i in range(NBANDS):
                row0 = i * HB + t * R
                dstv = out_ap[b, :, row0 : row0 + R, :].rearrange(
                    "c (s j) x -> j c s x", s=S, j=NCOLG
                )
                for j in range(NCOLG):
                    src = out_tiles[i][32 * j : 32 * j + 32, :].rearrange(
                        "c (s x) -> c s x", x=W
                    )
                    nc.sync.dma_start(out=dstv[j], in_=src)


def build_conv_nc(B, C, H, W, R=16, passes=1, do_mm=True):
    nc = bass.Bass("TRN2", target_bir_lowering=False, debug=False)
    x = nc.declare_dram_parameter("x", [B, C, H, W], F32, isOutput=False)
    w = nc.declare_dram_parameter("kernels_t", [C, KH * KW * C], F32, isOutput=False)
    out = nc.declare_dram_parameter("out", [B, C, H, W], F32, isOutput=True)
    with tile.TileContext(nc) as tc:
        with ExitStack() as ctx:
            emit_conv(nc, tc, ctx, x[:], w[:], out[:], B, C, H, W, R=R, do_mm=do_mm)
        # extra timing-probe passes into a scratch DRAM tensor (own pool
        # scope so SBUF is reused)
        for p in range(1, passes):
            scratch = nc.dram_tensor(f"scratch{p}", [B, C, H, W], F32)
            with ExitStack() as ctx:
                emit_conv(
                    nc, tc, ctx, x[:], w[:], scratch[:], B, C, H, W, R=R,
                    prefix=f"p{p}_", do_mm=do_mm,
                )
    split_multi_waits(nc, cap=1)
    return nc


_NC_CACHE = {}


def _get_nc():
    key = (FULL_B // N_CORES, FULL_C, FULL_H, FULL_W)
    if key not in _NC_CACHE:
        _NC_CACHE[key] = build_conv_nc(*key)
    return _NC_CACHE[key]


def host_weights(kernels: np.ndarray) -> np.ndarray:
    # [oc, ic, kh, kw] -> [ic, (kh kw oc)] contiguous, so the weight DMA is
    # a plain 2D copy.
    return np.ascontiguousarray(kernels.transpose(1, 2, 3, 0).reshape(32, -1))


def kernel(x: np.ndarray, kernels: np.ndarray) -> np.ndarray:
    assert x.shape == (FULL_B, FULL_C, FULL_H, FULL_W), x.shape
    nc = _get_nc()
    bl = FULL_B // N_CORES
    wt = host_weights(np.asarray(kernels, dtype=np.float32))
    xs = np.asarray(x, dtype=np.float32)
    in_maps = [
        {"x": xs[i * bl : (i + 1) * bl], "kernels_t": wt} for i in range(N_CORES)
    ]
    res = run_bass_kernel_spmd(nc, in_maps, list(range(N_CORES))).results
    out = np.concatenate([res[i]["out"] for i in range(N_CORES)], axis=0)
    return out.astype(np.float32, copy=False)
